# revision 37
# baseline (speedup 1.0000x reference)
"""BiAffine attention kernel for Trainium2, 8 NeuronCores.

Problem: b=8, n1=n2=2048, h=1024 (fp32)
  S2_h = S2 @ W1.T ; scores1 = S1 @ S2_h.T ; attn1 = softmax(scores1) ; O1 = attn1 @ S2
  S1_h = S1 @ W2.T ; scores2 = S2 @ S1_h.T ; attn2 = softmax(scores2) ; O2 = attn2 @ S1

Reformulated (per batch):
  scores1 = (S1 @ W1) @ S2^T        scores2 = (S2 @ W2) @ S1^T

Sharding: data-parallel over batch, 1 batch per core (8 cores).

v3 design: all matmuls keep an f32-family ifmap so every matmul stays
self-loading and --enable-ldw-opt=true remains legal (standalone
InstLdweights, produced for any non-f32 ifmap, is incompatible and
costs ~50ns/matmul when ldw-opt is disabled).

SBUF residents (per core): kT slots (4 group tiles, 64KB/part) hold s2T
during A1 and s1T during A2 (reloaded from HBM spills at the stage
boundary, overlapped); vnat (64KB/part) holds the AV values (S2 natural
then S1 natural, refilled by direct DMA). W-products are computed
fused per 512-column chunk (qw ring in SBUF): Wprod1 from freshly
transposed qTs chunks, Wprod2 from the still-resident s2T groups with
the result spilled to HBM and streamed back during A2.

All transposes use an fp32r identity as the moving operand (1.5
cyc/row). PSUM->SBUF copies run on DVE/ACT (Pool cannot touch PSUM).
"""

import sys

sys.path.insert(0, "/opt/trn_rl_repo")

import numpy as np

import concourse.bass as bass
import concourse.tile as tile
import concourse.mybir as mybir
from concourse import masks
from concourse.vector_clock import ScopedClock
import concourse.bass_utils as _bu

_orig_run_command = _bu.run_command


def _run_command_ldwopt(argv, **kw):
    argv = ["--enable-ldw-opt=true" if a == "--enable-ldw-opt=false" else a
            for a in argv]
    return _orig_run_command(argv, **kw)


_bu.run_command = _run_command_ldwopt

F32 = mybir.dt.float32
F32R = mybir.dt.float32r
BF16 = mybir.dt.bfloat16

P = 128            # partitions
H = 1024           # hidden
N = 2048           # sequence (n1 == n2)
KB = H // P        # 8 k-blocks of 128
MT = N // P        # 16 row tiles of 128
NCH = N // 512     # 4 column chunks of 512
CW = 512           # chunk width
AFT = mybir.ActivationFunctionType
AXX = mybir.AxisListType.X


class _TC(tile.TileContext):
    """TileContext for a walrus build that accepts at most ONE sync wait per
    instruction (2 on EventSemaphore): splits the final drain's waits, and
    runs a post-pass hoisting excess body waits into EventSemaphore carriers.
    """

    def _cap_waits(self):
        nc = self.nc
        for bbw in nc.bb_map.values():
            bb = bbw.bb
            insts = bb.instructions
            out = []
            changed = False
            for inst in insts:
                si = inst.sync_info
                cap = 2 if inst.opcode == "EventSemaphore" else 1
                if si is not None and len(si.on_wait) > cap:
                    waits = list(si.on_wait)
                    extra, keep = waits[:-cap], waits[-cap:]
                    while extra:
                        batch, extra = extra[:2], extra[2:]
                        carrier = mybir.InstEventSemaphore(
                            name=nc.get_next_instruction_name(),
                            ins=[], outs=[], engine=inst.engine,
                            sync_info=mybir.SyncInfo(on_wait=batch, on_update=[]),
                        )
                        out.append(carrier)
                    inst.sync_info = mybir.SyncInfo(
                        on_wait=keep, on_update=list(si.on_update))
                    changed = True
                out.append(inst)
            if changed:
                bb.instructions = out

    def _drain_and_barrier(self, tick_clock, wait_clock):
        self._cap_waits()
        nc = self.nc
        dummy = mybir.InstDrain(
            name="dummy_drain_waits", ins=[], outs=[], engine=mybir.EngineType.SP
        )
        wait_clock.add_sem_waits(dummy, ScopedClock({None: tick_clock.global_clock}))
        waits = list(dummy.sync_info.on_wait) if dummy.sync_info else []
        handles = {h.name: h for h in self.sems.allocated().values()}
        for w in waits:
            assert w.sync_type == "semaphore", w
            h = handles.get(w.ant_name)
            assert h is not None, (w.ant_name, sorted(handles))
            nc.sync.wait_ge(h, w.wait_value)
        nc.sync.drain()
        nc.all_engine_barrier()
        assert self.sems is not None
        popped = nc._tile_sem_poison_stack.pop()
        assert popped is self._sem_poison
        nc.clear_and_free_semaphores(list(self.sems.allocated().values()))
        nc.all_engine_barrier()


def _emit(tc, io, pools, last=False):
    nc = tc.nc
    (identR, kT, vnat, mpool, st_pool, ptp_pool, ps_pool, po_pool) = pools
    (S1, S2, W1, W2, O1, O2, s1T_d, s2T_d, s2wT_d) = io

    def t1_tile(i, qTs):
        # transpose one 128-row tile of S1 into the chunk-local qTs buffer
        xt = mpool.tile([P, H], F32R, tag="xt", bufs=2)
        nc.scalar.dma_start(out=xt[:], in_=S1.bitcast(F32R)[i * P:(i + 1) * P, :])
        r = i % 4
        for half in range(2):
            pt = ptp_pool.tile([P, CW], F32R, tag="ptp")
            for j in range(4):
                kb = half * 4 + j
                nc.tensor.transpose(pt[:, j * P:(j + 1) * P],
                                    xt[:, kb * P:(kb + 1) * P], identR[:])
            dst = qTs[:, half * 4:(half + 1) * 4, r * P:(r + 1) * P]
            src_ap = pt[:].rearrange("p (a b) -> p a b", a=4)
            if half == 0:
                nc.vector.tensor_copy(out=dst, in_=src_ap)
            else:
                nc.scalar.activation(dst, src_ap, AFT.Copy)

    def wprod(Wv, src, qw):
        # qw[p, hb, m] = sum_k W[k, hb*P+p] * src[k, m]
        for hp in range(4):
            wt = mpool.tile([P, KB, 2 * P], F32R, tag="wt", bufs=2)
            nc.sync.dma_start(out=wt[:],
                              in_=Wv[:, :, hp * 2 * P:(hp + 1) * 2 * P])
            for s in range(2):
                hb = hp * 2 + s
                pw = ptp_pool.tile([P, CW], F32, tag="ptp")
                for kb in range(KB):
                    nc.tensor.matmul(
                        pw[:],
                        lhsT=wt[:, kb, s * P:(s + 1) * P],
                        rhs=src[:, kb, :],
                        start=(kb == 0), stop=(kb == KB - 1),
                    )
                if hb % 2 == 0:
                    nc.vector.tensor_copy(out=qw[:, hb, :], in_=pw[:])
                else:
                    nc.scalar.activation(qw[:, hb, :], pw[:], AFT.Copy)

    def scores_softmax(qw, mt, kTg):
        ps = ps_pool.tile([P, N], F32, tag="ps")
        cmx = st_pool.tile([P, NCH], F32, tag="cmx")
        for ck in range(NCH):
            for kb in range(KB):
                nc.tensor.matmul(
                    ps[:, ck * CW:(ck + 1) * CW],
                    lhsT=qw[:, kb, mt * P:(mt + 1) * P],
                    rhs=kTg[ck][:, kb, :],
                    start=(kb == 0), stop=(kb == KB - 1),
                )
            nc.vector.reduce_max(out=cmx[:, ck:ck + 1],
                                 in_=ps[:, ck * CW:(ck + 1) * CW], axis=AXX)
        nmx = st_pool.tile([P, 1], F32, tag="nmx")
        nc.vector.reduce_max(out=nmx[:], in_=cmx[:], axis=AXX, negate=True)
        attn = mpool.tile([P, N], F32R, tag="attn", bufs=2)
        sumc = st_pool.tile([P, NCH], F32, tag="sumc")
        for ck in range(NCH):
            nc.scalar.activation(attn[:, ck * CW:(ck + 1) * CW],
                                 ps[:, ck * CW:(ck + 1) * CW], AFT.Exp,
                                 bias=nmx[:], accum_out=sumc[:, ck:ck + 1])
        sume = st_pool.tile([P, 1], F32, tag="sume")
        nc.vector.reduce_sum(out=sume[:], in_=sumc[:], axis=AXX)
        rec = st_pool.tile([P, 1], F32, tag="rec")
        nc.vector.reciprocal(rec[:], sume[:])
        return attn, rec

    def finish(attn, rec, g, O):
        aT = mpool.tile([P, MT, P], F32R, tag="aT", bufs=1)
        po = po_pool.tile([P, H], F32, tag="po")
        for q in range(4):
            pt = ptp_pool.tile([P, CW], F32R, tag="ptp")
            for j in range(4):
                nt = q * 4 + j
                nc.tensor.transpose(pt[:, j * P:(j + 1) * P],
                                    attn[:, nt * P:(nt + 1) * P], identR[:])
            nc.vector.tensor_copy(
                out=aT[:, q * 4:(q + 1) * 4, :],
                in_=pt[:].rearrange("p (a b) -> p a b", a=4))
            for nt in range(q * 4, q * 4 + 4):
                for hc in range(2):
                    nc.tensor.matmul(
                        po[:, hc * CW:(hc + 1) * CW],
                        lhsT=aT[:, nt, :],
                        rhs=vnat[:, nt, hc * CW:(hc + 1) * CW],
                        start=(nt == 0), stop=(nt == MT - 1),
                    )
        ot = mpool.tile([P, H], F32, tag="ot", bufs=2)
        nc.scalar.activation(ot[:], po[:], AFT.Copy, scale=rec[:])
        nc.scalar.dma_start(out=O[g * P:(g + 1) * P, :], in_=ot[:])

    Wv1 = W1.bitcast(F32R).rearrange("(kb p) h -> p kb h", p=P)
    Wv2 = W2.bitcast(F32R).rearrange("(kb p) h -> p kb h", p=P)

    # ---------------- A1: kT slots hold s2T; vnat holds S2 ----------------
    qTs = mpool.tile([P, KB, CW], F32R, tag="qTs", bufs=1)
    for i in range(4):
        t1_tile(i, qTs)
    prev = None
    for c in range(NCH):
        # spill this chunk of s1T for A2's kT reload (the kT slots still
        # hold s2T, so A2 must stream s1T back from HBM)
        nc.scalar.dma_start(out=s1T_d[:, :, c * CW:(c + 1) * CW], in_=qTs[:])
        qw = mpool.tile([P, KB, CW], F32R, tag="qw", bufs=1)
        wprod(Wv1, qTs, qw)
        qTs_next = None
        for mt in range(4):
            g = c * 4 + mt
            attn, rec = scores_softmax(qw, mt, kT)
            if prev is not None:
                finish(*prev, O1)
            if mt == 0:
                # Wprod2 here: spreads the W2 pair loads away from the next
                # chunk's W1 pairs, and gives exp(0) time before finish(0).
                # qw2 reuses the qTs slot (Wprod1+spill done with it).
                qw2 = mpool.tile([P, KB, CW], F32R, tag="qTs", bufs=1)
                wprod(Wv2, kT[c][:], qw2)
                nc.scalar.dma_start(out=s2wT_d[:, :, c * CW:(c + 1) * CW],
                                    in_=qw2[:])
            if c < NCH - 1 and mt >= 2:
                if mt == 2:
                    qTs_next = mpool.tile([P, KB, CW], F32R, tag="qTs",
                                          bufs=1)
                t1_tile(4 * (c + 1) + 2 * (mt - 2), qTs_next)
                t1_tile(4 * (c + 1) + 2 * (mt - 2) + 1, qTs_next)
            prev = (attn, rec, g)
        if qTs_next is not None:
            qTs = qTs_next
    finish(*prev, O1)

    # ------------- boundary: swap kT -> s1T, vnat -> S1 natural ------------
    for g in range(NCH):
        nc.sync.dma_start(out=kT[g][:], in_=s1T_d[:, :, g * CW:(g + 1) * CW])
    for i in range(MT):
        nc.scalar.dma_start(out=vnat[:, i, :],
                            in_=S1.bitcast(F32R)[i * P:(i + 1) * P, :])

    # ---------------- A2: kT slots hold s1T; vnat holds S1 ----------------
    prev = None
    for c in range(NCH):
        qw = mpool.tile([P, KB, CW], F32R, tag="qw", bufs=1)
        # piece-wise load: scores(mt) only needs columns mt*P:(mt+1)*P, so
        # the first tile starts after ~1/4 of the chunk's qw has landed
        for mt in range(4):
            nc.sync.dma_start(
                out=qw[:, :, mt * P:(mt + 1) * P],
                in_=s2wT_d[:, :, c * CW + mt * P:c * CW + (mt + 1) * P])
        for mt in range(4):
            g = c * 4 + mt
            attn, rec = scores_softmax(qw, mt, kT)
            if prev is not None:
                finish(*prev, O2)
            prev = (attn, rec, g)
    finish(*prev, O2)

    # ------------- boundary 2: restore kT -> s2T, vnat -> S2 ---------------
    if not last:
        for g in range(NCH):
            nc.sync.dma_start(out=kT[g][:],
                              in_=s2T_d[:, :, g * CW:(g + 1) * CW])
        for i in range(MT):
            nc.scalar.dma_start(out=vnat[:, i, :],
                                in_=S2.bitcast(F32R)[i * P:(i + 1) * P, :])


def _emit_prologue(tc, io, pools):
    nc = tc.nc
    (identR, kT, vnat, mpool, st_pool, ptp_pool, ps_pool, po_pool) = pools
    (S1, S2, W1, W2, O1, O2, s1T_d, s2T_d, s2wT_d) = io
    # build s2T into the kT slots, spill each group to s2T_d, fill vnat <- S2
    for i in range(MT):
        xt = mpool.tile([P, H], F32R, tag="xt", bufs=2)
        nc.scalar.dma_start(out=xt[:], in_=S2.bitcast(F32R)[i * P:(i + 1) * P, :])
        nc.sync.dma_start(out=vnat[:, i, :],
                          in_=S2.bitcast(F32R)[i * P:(i + 1) * P, :])
        g, r = divmod(i, 4)
        for half in range(2):
            pt = ptp_pool.tile([P, CW], F32R, tag="ptp")
            for j in range(4):
                kb = half * 4 + j
                nc.tensor.transpose(pt[:, j * P:(j + 1) * P],
                                    xt[:, kb * P:(kb + 1) * P], identR[:])
            dst = kT[g][:, half * 4:(half + 1) * 4, r * P:(r + 1) * P]
            src_ap = pt[:].rearrange("p (a b) -> p a b", a=4)
            if half == 0:
                nc.vector.tensor_copy(out=dst, in_=src_ap)
            else:
                nc.scalar.activation(dst, src_ap, AFT.Copy)
        if r == 3:
            nc.scalar.dma_start(out=s2T_d[:, :, g * CW:(g + 1) * CW],
                                in_=kT[g][:])


def build(reps=1, loop=None):
    nc = bass.Bass(name="biaffine", dynamic_dma_scratch_size=2048)
    S1 = nc.dram_tensor("S1", (N, H), F32, kind="ExternalInput")[:]
    S2 = nc.dram_tensor("S2", (N, H), F32, kind="ExternalInput")[:]
    W1 = nc.dram_tensor("W1", (H, H), F32, kind="ExternalInput")[:]
    W2 = nc.dram_tensor("W2", (H, H), F32, kind="ExternalInput")[:]
    O1 = nc.dram_tensor("O1", (N, H), F32, kind="ExternalOutput")[:]
    O2 = nc.dram_tensor("O2", (N, H), F32, kind="ExternalOutput")[:]
    s1T_d = nc.dram_tensor("s1T_sp", (P, KB, N), F32R, kind="Internal")[:]
    s2T_d = nc.dram_tensor("s2T_sp", (P, KB, N), F32R, kind="Internal")[:]
    s2wT_d = nc.dram_tensor("s2wT_sp", (P, KB, N), F32R, kind="Internal")[:]
    io = (S1, S2, W1, W2, O1, O2, s1T_d, s2T_d, s2wT_d)

    with _TC(nc) as tc:
        with tc.tile_pool(name="consts", bufs=1) as consts, \
             tc.tile_pool(name="ktp", bufs=1) as ktp, \
             tc.tile_pool(name="vp", bufs=1) as vp, \
             tc.tile_pool(name="main", bufs=2) as mpool, \
             tc.tile_pool(name="st", bufs=4) as st_pool, \
             tc.tile_pool(name="ptp", bufs=2, space="PSUM") as ptp_pool, \
             tc.tile_pool(name="ps", bufs=1, space="PSUM") as ps_pool, \
             tc.tile_pool(name="po", bufs=1, space="PSUM") as po_pool:
            identF = consts.tile([P, P], F32)
            masks.make_identity(nc, identF[:])
            identR = consts.tile([P, P], F32R)
            nc.vector.tensor_copy(out=identR[:], in_=identF[:])
            kT = [ktp.tile([P, KB, CW], F32R, tag=f"g{g}", name=f"kT{g}")
                  for g in range(NCH)]
            vnat = vp.tile([P, MT, H], F32R)
            pools = (identR, kT, vnat, mpool, st_pool, ptp_pool, ps_pool,
                     po_pool)
            _emit_prologue(tc, io, pools)
            if loop is not None:
                with tc.For_i(0, loop, 1):
                    _emit(tc, io, pools)
            else:
                for r in range(reps):
                    _emit(tc, io, pools, last=(r == reps - 1))
    return nc


_nc_cache = {}


def _get_nc(reps=1):
    if reps not in _nc_cache:
        _nc_cache[reps] = build(reps)
    return _nc_cache[reps]


def run_on_cores(inputs, reps=1):
    from concourse.bass_utils import run_bass_kernel_spmd

    nc = _get_nc(reps)
    S1 = np.asarray(inputs["S1"], dtype=np.float32)
    S2 = np.asarray(inputs["S2"], dtype=np.float32)
    W1 = np.ascontiguousarray(np.asarray(inputs["W1"], dtype=np.float32))
    W2 = np.ascontiguousarray(np.asarray(inputs["W2"], dtype=np.float32))
    b = S1.shape[0]
    assert b == 8
    in_maps = [
        {
            "S1": np.ascontiguousarray(S1[i]),
            "S2": np.ascontiguousarray(S2[i]),
            "W1": W1,
            "W2": W2,
        }
        for i in range(b)
    ]
    res = run_bass_kernel_spmd(nc, in_maps, core_ids=list(range(b)))
    O1 = np.stack([res.results[i]["O1"] for i in range(b)])
    O2 = np.stack([res.results[i]["O2"] for i in range(b)])
    return O1, O2


def kernel(**inputs):
    O1, O2 = run_on_cores(inputs, reps=1)
    return O1.astype(np.float32), O2.astype(np.float32)


# revision 38
# speedup vs baseline: 1.1537x; 1.1537x over previous
"""BiAffine attention kernel for Trainium2, 8 NeuronCores.

Problem: b=8, n1=n2=2048, h=1024 (fp32)
  S2_h = S2 @ W1.T ; scores1 = S1 @ S2_h.T ; attn1 = softmax(scores1) ; O1 = attn1 @ S2
  S1_h = S1 @ W2.T ; scores2 = S2 @ S1_h.T ; attn2 = softmax(scores2) ; O2 = attn2 @ S1

Reformulated (per batch):
  scores1 = (S1 @ W1) @ S2^T        scores2 = (S2 @ W2) @ S1^T

Sharding: data-parallel over batch, 1 batch per core (8 cores).

v3 design: all matmuls keep an f32-family ifmap so every matmul stays
self-loading and --enable-ldw-opt=true remains legal (standalone
InstLdweights, produced for any non-f32 ifmap, is incompatible and
costs ~50ns/matmul when ldw-opt is disabled).

SBUF residents (per core): kT slots (4 group tiles, 64KB/part) hold s2T
during A1 and s1T during A2 (reloaded from HBM spills at the stage
boundary, overlapped); vnat (64KB/part) holds the AV values (S2 natural
then S1 natural, refilled by direct DMA). W-products are computed
fused per 512-column chunk (qw ring in SBUF): Wprod1 from freshly
transposed qTs chunks, Wprod2 from the still-resident s2T groups with
the result spilled to HBM and streamed back during A2.

All transposes use an fp32r identity as the moving operand (1.5
cyc/row). PSUM->SBUF copies run on DVE/ACT (Pool cannot touch PSUM).
"""

import sys

sys.path.insert(0, "/opt/trn_rl_repo")

import numpy as np

import concourse.bass as bass
import concourse.tile as tile
import concourse.mybir as mybir
from concourse import masks
from concourse.vector_clock import ScopedClock
import concourse.bass_utils as _bu

_orig_run_command = _bu.run_command


def _run_command_ldwopt(argv, **kw):
    argv = ["--enable-ldw-opt=true" if a == "--enable-ldw-opt=false" else a
            for a in argv]
    return _orig_run_command(argv, **kw)


_bu.run_command = _run_command_ldwopt

F32 = mybir.dt.float32
F32R = mybir.dt.float32r
BF16 = mybir.dt.bfloat16

P = 128            # partitions
H = 1024           # hidden
N = 2048           # sequence (n1 == n2)
KB = H // P        # 8 k-blocks of 128
MT = N // P        # 16 row tiles of 128
NCH = N // 512     # 4 column chunks of 512
CW = 512           # chunk width
AFT = mybir.ActivationFunctionType
AXX = mybir.AxisListType.X


class _TC(tile.TileContext):
    """TileContext for a walrus build that accepts at most ONE sync wait per
    instruction (2 on EventSemaphore): splits the final drain's waits, and
    runs a post-pass hoisting excess body waits into EventSemaphore carriers.
    """

    def _cap_waits(self):
        nc = self.nc
        for bbw in nc.bb_map.values():
            bb = bbw.bb
            insts = bb.instructions
            out = []
            changed = False
            for inst in insts:
                si = inst.sync_info
                cap = 2 if inst.opcode == "EventSemaphore" else 1
                if si is not None and len(si.on_wait) > cap:
                    waits = list(si.on_wait)
                    extra, keep = waits[:-cap], waits[-cap:]
                    while extra:
                        batch, extra = extra[:2], extra[2:]
                        carrier = mybir.InstEventSemaphore(
                            name=nc.get_next_instruction_name(),
                            ins=[], outs=[], engine=inst.engine,
                            sync_info=mybir.SyncInfo(on_wait=batch, on_update=[]),
                        )
                        out.append(carrier)
                    inst.sync_info = mybir.SyncInfo(
                        on_wait=keep, on_update=list(si.on_update))
                    changed = True
                out.append(inst)
            if changed:
                bb.instructions = out

    def _drain_and_barrier(self, tick_clock, wait_clock):
        self._cap_waits()
        nc = self.nc
        dummy = mybir.InstDrain(
            name="dummy_drain_waits", ins=[], outs=[], engine=mybir.EngineType.SP
        )
        wait_clock.add_sem_waits(dummy, ScopedClock({None: tick_clock.global_clock}))
        waits = list(dummy.sync_info.on_wait) if dummy.sync_info else []
        handles = {h.name: h for h in self.sems.allocated().values()}
        for w in waits:
            assert w.sync_type == "semaphore", w
            h = handles.get(w.ant_name)
            assert h is not None, (w.ant_name, sorted(handles))
            nc.sync.wait_ge(h, w.wait_value)
        nc.sync.drain()
        nc.all_engine_barrier()
        assert self.sems is not None
        popped = nc._tile_sem_poison_stack.pop()
        assert popped is self._sem_poison
        nc.clear_and_free_semaphores(list(self.sems.allocated().values()))
        nc.all_engine_barrier()


def _emit(tc, io, pools, last=False):
    nc = tc.nc
    (identR, kT, vnat, mpool, st_pool, ptp_pool, ps_pool, po_pool) = pools
    (S1, S2, W1, W2, O1, O2, s1T_d, s2T_d, s2wT_d) = io

    def t1_tile(i, qTs):
        # transpose one 128-row tile of S1 into the chunk-local qTs buffer
        xt = mpool.tile([P, H], F32R, tag="xt", bufs=2)
        nc.scalar.dma_start(out=xt[:], in_=S1.bitcast(F32R)[i * P:(i + 1) * P, :])
        r = i % 4
        for half in range(2):
            pt = ptp_pool.tile([P, CW], F32R, tag="ptp")
            for j in range(4):
                kb = half * 4 + j
                nc.tensor.transpose(pt[:, j * P:(j + 1) * P],
                                    xt[:, kb * P:(kb + 1) * P], identR[:])
            dst = qTs[:, half * 4:(half + 1) * 4, r * P:(r + 1) * P]
            src_ap = pt[:].rearrange("p (a b) -> p a b", a=4)
            if half == 0:
                nc.vector.tensor_copy(out=dst, in_=src_ap)
            else:
                nc.scalar.activation(dst, src_ap, AFT.Copy)

    def wprod(Wv, src, qw):
        # qw[p, hb, m] = sum_k W[k, hb*P+p] * src[k, m]
        for hp in range(4):
            wt = mpool.tile([P, KB, 2 * P], F32R, tag="wt", bufs=2)
            nc.sync.dma_start(out=wt[:],
                              in_=Wv[:, :, hp * 2 * P:(hp + 1) * 2 * P])
            for s in range(2):
                hb = hp * 2 + s
                pw = ptp_pool.tile([P, CW], F32, tag="ptp")
                for kb in range(KB):
                    nc.tensor.matmul(
                        pw[:],
                        lhsT=wt[:, kb, s * P:(s + 1) * P],
                        rhs=src[:, kb, :],
                        start=(kb == 0), stop=(kb == KB - 1),
                    )
                if hb % 2 == 0:
                    nc.vector.tensor_copy(out=qw[:, hb, :], in_=pw[:])
                else:
                    nc.scalar.activation(qw[:, hb, :], pw[:], AFT.Copy)

    def scores_softmax(qw, mt, kTg):
        ps = ps_pool.tile([P, N], F32, tag="ps")
        cmx = st_pool.tile([P, NCH], F32, tag="cmx")
        for ck in range(NCH):
            for kb in range(KB):
                nc.tensor.matmul(
                    ps[:, ck * CW:(ck + 1) * CW],
                    lhsT=qw[:, kb, mt * P:(mt + 1) * P],
                    rhs=kTg[ck][:, kb, :],
                    start=(kb == 0), stop=(kb == KB - 1),
                )
            nc.vector.reduce_max(out=cmx[:, ck:ck + 1],
                                 in_=ps[:, ck * CW:(ck + 1) * CW], axis=AXX)
        nmx = st_pool.tile([P, 1], F32, tag="nmx")
        nc.vector.reduce_max(out=nmx[:], in_=cmx[:], axis=AXX, negate=True)
        attn = mpool.tile([P, N], F32R, tag="attn", bufs=2)
        sumc = st_pool.tile([P, NCH], F32, tag="sumc")
        for ck in range(NCH):
            nc.scalar.activation(attn[:, ck * CW:(ck + 1) * CW],
                                 ps[:, ck * CW:(ck + 1) * CW], AFT.Exp,
                                 bias=nmx[:], accum_out=sumc[:, ck:ck + 1])
        sume = st_pool.tile([P, 1], F32, tag="sume")
        nc.vector.reduce_sum(out=sume[:], in_=sumc[:], axis=AXX)
        rec = st_pool.tile([P, 1], F32, tag="rec")
        nc.vector.reciprocal(rec[:], sume[:])
        return attn, rec

    def finish(attn, rec, g, O):
        aT = mpool.tile([P, MT, P], F32R, tag="aT", bufs=1)
        po = po_pool.tile([P, H], F32, tag="po")
        for q in range(4):
            pt = ptp_pool.tile([P, CW], F32R, tag="ptp")
            for j in range(4):
                nt = q * 4 + j
                nc.tensor.transpose(pt[:, j * P:(j + 1) * P],
                                    attn[:, nt * P:(nt + 1) * P], identR[:])
            nc.vector.tensor_copy(
                out=aT[:, q * 4:(q + 1) * 4, :],
                in_=pt[:].rearrange("p (a b) -> p a b", a=4))
            for nt in range(q * 4, q * 4 + 4):
                for hc in range(2):
                    nc.tensor.matmul(
                        po[:, hc * CW:(hc + 1) * CW],
                        lhsT=aT[:, nt, :],
                        rhs=vnat[:, nt, hc * CW:(hc + 1) * CW],
                        start=(nt == 0), stop=(nt == MT - 1),
                    )
        ot = mpool.tile([P, H], F32, tag="ot", bufs=2)
        nc.scalar.activation(ot[:], po[:], AFT.Copy, scale=rec[:])
        nc.scalar.dma_start(out=O[g * P:(g + 1) * P, :], in_=ot[:])

    Wv1 = W1.bitcast(F32R).rearrange("(kb p) h -> p kb h", p=P)
    Wv2 = W2.bitcast(F32R).rearrange("(kb p) h -> p kb h", p=P)

    # ---------------- A1: kT slots hold s2T; vnat holds S2 ----------------
    qTs = mpool.tile([P, KB, CW], F32R, tag="qTs", bufs=1)
    for i in range(4):
        t1_tile(i, qTs)
    prev = None
    for c in range(NCH):
        # spill this chunk of s1T for A2's kT reload (the kT slots still
        # hold s2T, so A2 must stream s1T back from HBM)
        nc.scalar.dma_start(out=s1T_d[:, :, c * CW:(c + 1) * CW], in_=qTs[:])
        qw = mpool.tile([P, KB, CW], F32R, tag="qw", bufs=1)
        wprod(Wv1, qTs, qw)
        # Wprod2 immediately after Wprod1; qw2 reuses the qTs slot
        qw2 = mpool.tile([P, KB, CW], F32R, tag="qTs", bufs=1)
        wprod(Wv2, kT[c][:], qw2)
        nc.scalar.dma_start(out=s2wT_d[:, :, c * CW:(c + 1) * CW], in_=qw2[:])
        qTs_next = None
        for mt in range(4):
            g = c * 4 + mt
            attn, rec = scores_softmax(qw, mt, kT)
            if prev is not None:
                finish(*prev, O1)
            if c < NCH - 1 and mt >= 2:
                if mt == 2:
                    qTs_next = mpool.tile([P, KB, CW], F32R, tag="qTs",
                                          bufs=1)
                t1_tile(4 * (c + 1) + 2 * (mt - 2), qTs_next)
                t1_tile(4 * (c + 1) + 2 * (mt - 2) + 1, qTs_next)
            prev = (attn, rec, g)
        if qTs_next is not None:
            qTs = qTs_next
    finish(*prev, O1)

    # ------------- boundary: swap kT -> s1T, vnat -> S1 natural ------------
    for g in range(NCH):
        nc.sync.dma_start(out=kT[g][:], in_=s1T_d[:, :, g * CW:(g + 1) * CW])
    for i in range(MT):
        nc.scalar.dma_start(out=vnat[:, i, :],
                            in_=S1.bitcast(F32R)[i * P:(i + 1) * P, :])

    # ---------------- A2: kT slots hold s1T; vnat holds S1 ----------------
    prev = None
    for c in range(NCH):
        qw = mpool.tile([P, KB, CW], F32R, tag="qw", bufs=1)
        # piece-wise load: scores(mt) only needs columns mt*P:(mt+1)*P, so
        # the first tile starts after ~1/4 of the chunk's qw has landed
        for mt in range(4):
            nc.sync.dma_start(
                out=qw[:, :, mt * P:(mt + 1) * P],
                in_=s2wT_d[:, :, c * CW + mt * P:c * CW + (mt + 1) * P])
        for mt in range(4):
            g = c * 4 + mt
            attn, rec = scores_softmax(qw, mt, kT)
            if prev is not None:
                finish(*prev, O2)
            prev = (attn, rec, g)
    finish(*prev, O2)

    # ------------- boundary 2: restore kT -> s2T, vnat -> S2 ---------------
    if not last:
        for g in range(NCH):
            nc.sync.dma_start(out=kT[g][:],
                              in_=s2T_d[:, :, g * CW:(g + 1) * CW])
        for i in range(MT):
            nc.scalar.dma_start(out=vnat[:, i, :],
                                in_=S2.bitcast(F32R)[i * P:(i + 1) * P, :])


def _emit_prologue(tc, io, pools):
    nc = tc.nc
    (identR, kT, vnat, mpool, st_pool, ptp_pool, ps_pool, po_pool) = pools
    (S1, S2, W1, W2, O1, O2, s1T_d, s2T_d, s2wT_d) = io
    # build s2T into the kT slots, spill each group to s2T_d, fill vnat <- S2
    for i in range(MT):
        xt = mpool.tile([P, H], F32R, tag="xt", bufs=2)
        nc.scalar.dma_start(out=xt[:], in_=S2.bitcast(F32R)[i * P:(i + 1) * P, :])
        nc.sync.dma_start(out=vnat[:, i, :],
                          in_=S2.bitcast(F32R)[i * P:(i + 1) * P, :])
        g, r = divmod(i, 4)
        for half in range(2):
            pt = ptp_pool.tile([P, CW], F32R, tag="ptp")
            for j in range(4):
                kb = half * 4 + j
                nc.tensor.transpose(pt[:, j * P:(j + 1) * P],
                                    xt[:, kb * P:(kb + 1) * P], identR[:])
            dst = kT[g][:, half * 4:(half + 1) * 4, r * P:(r + 1) * P]
            src_ap = pt[:].rearrange("p (a b) -> p a b", a=4)
            if half == 0:
                nc.vector.tensor_copy(out=dst, in_=src_ap)
            else:
                nc.scalar.activation(dst, src_ap, AFT.Copy)
        if r == 3:
            nc.scalar.dma_start(out=s2T_d[:, :, g * CW:(g + 1) * CW],
                                in_=kT[g][:])


def build(reps=1, loop=None):
    nc = bass.Bass(name="biaffine", dynamic_dma_scratch_size=2048)
    S1 = nc.dram_tensor("S1", (N, H), F32, kind="ExternalInput")[:]
    S2 = nc.dram_tensor("S2", (N, H), F32, kind="ExternalInput")[:]
    W1 = nc.dram_tensor("W1", (H, H), F32, kind="ExternalInput")[:]
    W2 = nc.dram_tensor("W2", (H, H), F32, kind="ExternalInput")[:]
    O1 = nc.dram_tensor("O1", (N, H), F32, kind="ExternalOutput")[:]
    O2 = nc.dram_tensor("O2", (N, H), F32, kind="ExternalOutput")[:]
    s1T_d = nc.dram_tensor("s1T_sp", (P, KB, N), F32R, kind="Internal")[:]
    s2T_d = nc.dram_tensor("s2T_sp", (P, KB, N), F32R, kind="Internal")[:]
    s2wT_d = nc.dram_tensor("s2wT_sp", (P, KB, N), F32R, kind="Internal")[:]
    io = (S1, S2, W1, W2, O1, O2, s1T_d, s2T_d, s2wT_d)

    with _TC(nc) as tc:
        with tc.tile_pool(name="consts", bufs=1) as consts, \
             tc.tile_pool(name="ktp", bufs=1) as ktp, \
             tc.tile_pool(name="vp", bufs=1) as vp, \
             tc.tile_pool(name="main", bufs=2) as mpool, \
             tc.tile_pool(name="st", bufs=4) as st_pool, \
             tc.tile_pool(name="ptp", bufs=2, space="PSUM") as ptp_pool, \
             tc.tile_pool(name="ps", bufs=1, space="PSUM") as ps_pool, \
             tc.tile_pool(name="po", bufs=1, space="PSUM") as po_pool:
            identF = consts.tile([P, P], F32)
            masks.make_identity(nc, identF[:])
            identR = consts.tile([P, P], F32R)
            nc.vector.tensor_copy(out=identR[:], in_=identF[:])
            kT = [ktp.tile([P, KB, CW], F32R, tag=f"g{g}", name=f"kT{g}")
                  for g in range(NCH)]
            vnat = vp.tile([P, MT, H], F32R)
            pools = (identR, kT, vnat, mpool, st_pool, ptp_pool, ps_pool,
                     po_pool)
            _emit_prologue(tc, io, pools)
            if loop is not None:
                with tc.For_i(0, loop, 1):
                    _emit(tc, io, pools)
            else:
                for r in range(reps):
                    _emit(tc, io, pools, last=(r == reps - 1))
    return nc


_nc_cache = {}


def _get_nc(reps=1):
    if reps not in _nc_cache:
        _nc_cache[reps] = build(reps)
    return _nc_cache[reps]


def run_on_cores(inputs, reps=1):
    from concourse.bass_utils import run_bass_kernel_spmd

    nc = _get_nc(reps)
    S1 = np.asarray(inputs["S1"], dtype=np.float32)
    S2 = np.asarray(inputs["S2"], dtype=np.float32)
    W1 = np.ascontiguousarray(np.asarray(inputs["W1"], dtype=np.float32))
    W2 = np.ascontiguousarray(np.asarray(inputs["W2"], dtype=np.float32))
    b = S1.shape[0]
    assert b == 8
    in_maps = [
        {
            "S1": np.ascontiguousarray(S1[i]),
            "S2": np.ascontiguousarray(S2[i]),
            "W1": W1,
            "W2": W2,
        }
        for i in range(b)
    ]
    res = run_bass_kernel_spmd(nc, in_maps, core_ids=list(range(b)))
    O1 = np.stack([res.results[i]["O1"] for i in range(b)])
    O2 = np.stack([res.results[i]["O2"] for i in range(b)])
    return O1, O2


def kernel(**inputs):
    O1, O2 = run_on_cores(inputs, reps=1)
    return O1.astype(np.float32), O2.astype(np.float32)
